# revision 51
# baseline (speedup 1.0000x reference)
"""Trainium2 Bass kernel for the BiRNN LM problem.

Computation (per step t over SEQ=64):
    emb    = we[tok_t]                       [B=32, E=32]
    hidden = tanh([emb, hidden] @ i2h)       [B=32, H=16]
    out_t  = exp(hidden @ i2o)               [B=32, V=32000]
    out_t /= sum(out_t)                      (global sum over the whole slab)

Sharding: sequence dim across 8 cores, interleaved: core c produces output
steps {c + 8k, k=0..7}.  Each step's normalization sum is fully local to one
core => no collectives.  Every core replicates the (tiny) 64-step hidden
recurrence; per-core step selection uses partition_id()-based dynamic DRAM
slices, so all cores run one identical SPMD program.

Layout:
  * EMBH[k] [48, 33*32] f32: rows 0:32 hold transposed embeddings for steps
    32k..32k+31 (col block t-32k), rows 32:48 hold transposed hiddens, with
    h_t landed at col block t+1-32k by the tanh itself.  The recurrence is
    ONE matmul (lhsT=i2h [48,16]) + one tanh per step.
  * Output pass packs 4 steps x 32 batch onto the 128 partitions; the big
    matmul + exp + normalize + store all run in bf16 (except f32 psum/sums),
    which keeps PE at 1 cyc/col, DVE in 2x mode and halves the HBM writes.
    Final upconvert to f32 happens on the host (pure dtype cast).
  * Chain half 2 is emitted before group 0's exp macros: the Tile scheduler
    then naturally alternates tanh (serial chain) with exp macros on ACT,
    hiding half of the chain latency behind group-0 output compute.
"""

import sys
import numpy as np
import ml_dtypes

sys.path.insert(0, "/opt/trn_rl_repo")

import concourse.bass as bass
import concourse.bacc as bacc
import concourse.mybir as mybir
import concourse.tile as tile
from concourse.bass_types import DynSlice
from concourse.bass_utils import run_bass_kernel_spmd
from concourse.tile import add_dep_helper

F32 = mybir.dt.float32
F32R = mybir.dt.float32r
BF16 = mybir.dt.bfloat16
I32 = mybir.dt.int32
AF = mybir.ActivationFunctionType

SEQ, B, E, H, V = 64, 32, 32, 16, 32000
NCORES = 8
NGROUP = 2                   # groups of 4 steps (4*32 = 128 partitions)
GSTEP = 4
CHUNK = 512                  # matmul free dim (one PSUM bank of f32)
MACRO = 1536                 # ACT exp granularity (3 banks)
PIECE = 3072                 # mul + DMA granularity (2 macros)
ILOAD = 4000                 # i2o load slice (cols per DMA)


def build():
    nc = bacc.Bacc("TRN2", target_bir_lowering=False, debug=False,
                   num_devices=NCORES)

    tok_d = nc.dram_tensor("tokT", [128, 16], I32, kind="ExternalInput")
    h0_d = nc.dram_tensor("h0T", [H, B], F32, kind="ExternalInput")
    we_d = nc.dram_tensor("we", [V, E], F32, kind="ExternalInput")
    i2h_d = nc.dram_tensor("i2h", [E + H, H], F32, kind="ExternalInput")
    i2o_d = nc.dram_tensor("i2oB", [H, V], BF16, kind="ExternalInput")
    mask_d = nc.dram_tensor("mask4", [128, 4], F32, kind="ExternalInput")
    maskT_d = nc.dram_tensor("maskT4", [4, 128], F32, kind="ExternalInput")

    out_d = nc.dram_tensor("out", [NGROUP, 128, V], BF16, kind="ExternalOutput")
    # per-half hidden archives (bf16): h of output step t at col 32*(t-32k)
    harc = [nc.dram_tensor(f"harc{k}", [H, 32 * B], BF16, kind="Internal")
            for k in range(2)]

    pieces = [min(PIECE, V - k * PIECE) for k in range((V + PIECE - 1) // PIECE)]
    # macros: (piece, col within piece, width); every macro inside one piece
    grid = []
    col = 0
    while col < V:
        piece = col // PIECE
        w = min(MACRO, V - col, (piece + 1) * PIECE - col)
        grid.append((piece, col - piece * PIECE, w))
        col += w
    nmacro = len(grid)

    with tile.TileContext(nc) as tc:
        with (
            tc.tile_pool(name="const", bufs=1) as constp,
            tc.tile_pool(name="embg", bufs=5) as embgp,
            tc.tile_pool(name="grp", bufs=2) as grpp,
            tc.tile_pool(name="slab", bufs=len(pieces) + 3) as slabp,
            tc.tile_pool(name="pmm", bufs=2, space="PSUM") as pmmp,
            tc.tile_pool(name="phc", bufs=2, space="PSUM") as phcp,
        ):
            pid = nc.partition_id(
                engines=(mybir.EngineType.SP, mybir.EngineType.Pool))

            # dummy activation first: pulls ACT_TABLE_LOAD (exp/tanh set) to
            # t=0 instead of serializing it in front of the first real tanh.
            warm = constp.tile([1, 1], F32)
            nc.vector.memset(warm[:], 0)
            nc.scalar.activation(warm[:], warm[:], AF.Exp)

            # ---- constants / inputs to SBUF ----
            tok = constp.tile([128, 16], I32)
            nc.sync.dma_start(tok[:], tok_d.ap())
            i2h = constp.tile([E + H, H], F32)
            nc.sync.dma_start(i2h[:], i2h_d.ap())
            mask4 = constp.tile([128, 4], F32)
            nc.sync.dma_start(mask4[:], mask_d.ap())
            maskT4 = constp.tile([4, 128], F32)
            nc.sync.dma_start(maskT4[:], maskT_d.ap())

            # combined [emb; h] per half: blocks 0..32, h_t stored at block t+1
            embh = [constp.tile([E + H, 33 * B], F32, name=f"embh{k}")
                    for k in range(2)]
            nc.sync.dma_start(embh[0][E:E + H, 0:B], h0_d.ap())

            # ---- embedding gather + DVE 32x32 block transposes ----
            for j in range(16):
                eg = embgp.tile([128, E], F32, tag="eg")
                nc.gpsimd.indirect_dma_start(
                    out=eg[:], out_offset=None, in_=we_d.ap(),
                    in_offset=bass.IndirectOffsetOnAxis(ap=tok[:, j:j + 1], axis=0))
                for b in range(4):
                    nc.vector.transpose(
                        embh[j // 8][0:E, 128 * (j % 8) + 32 * b:
                                     128 * (j % 8) + 32 * (b + 1)],
                        eg[32 * b:32 * (b + 1), :])

            # bf16 staging copies of h (DVE, off the serial path): hstage[k]
            # col block s holds h of output step 32k+s.  The group archive
            # DMA reads hstage, so it depends only on the tanhs — not on the
            # embedding transposes that share the embh tiles.
            hstage = [constp.tile([H, 32 * B], BF16, name=f"hstage{k}")
                      for k in range(2)]

            def stage_copy(t):
                # after tanh of step t (t = 4a+3): copy steps 4a..4a+3
                k, a = t // 32, (t % 32) // 4
                if k == 0 and a == 7:
                    # h for steps 28..30 in embh[0] blocks 29..31; step 31's
                    # h in embh[1] block 0
                    nc.vector.tensor_copy(hstage[0][:, 28 * B:31 * B],
                                          embh[0][E:E + H, 29 * B:32 * B])
                    nc.vector.tensor_copy(hstage[0][:, 31 * B:32 * B],
                                          embh[1][E:E + H, 0:B])
                else:
                    nc.vector.tensor_copy(
                        hstage[k][:, 4 * a * B:(4 * a + 4) * B],
                        embh[k][E:E + H, (4 * a + 1) * B:(4 * a + 5) * B])

            # i2o [16, V] on partitions 0:16, sliced loads on the scalar
            # HWDGE queue (keeps the sync queue short for the group preps;
            # slices stay ahead of the exp pass)
            i2o = constp.tile([H, V], BF16)
            for c0 in range(0, V, ILOAD):
                cw = min(ILOAD, V - c0)
                nc.scalar.dma_start(i2o[:, c0:c0 + cw],
                                    i2o_d.ap()[:, c0:c0 + cw])

            # ---- recurrence: one matmul + one tanh per step ----
            def chain_steps(t0, t1):
                for t in range(t0, t1):
                    k, b = t // 32, t % 32
                    hp = phcp.tile([H, B], F32, space="PSUM", tag="hps")
                    nc.tensor.matmul(hp[:], i2h[:],
                                     embh[k][:, B * b:B * (b + 1)],
                                     start=True, stop=True)
                    dstk, dstb = (t + 1) // 32, (t + 1) % 32
                    if t == 63:
                        dstk, dstb = 1, 32   # park h_64 in embh[1] block 32
                    nc.scalar.activation(
                        embh[dstk][E:E + H, B * dstb:B * (dstb + 1)],
                        hp[:], AF.Tanh)
                    if t % 4 == 3:
                        stage_copy(t)
                    if t % 8 == 7:
                        # archive this 8-step slice now so only the last
                        # slice + one lhsT load remain on the group's
                        # critical path after the half completes
                        g, p = t // 32, (t % 32) // 8
                        nc.sync.dma_start(
                            harc[g].ap()[:, 8 * B * p:8 * B * (p + 1)],
                            hstage[g][:, 8 * B * p:8 * B * (p + 1)])

            def prep_group(g):
                # fetch this core's 4 steps with a dynamic-offset DMA read:
                # h of output step c+32g+8i sits at archive col 32*(c+8i),
                # c = partition_id (the archive slices were written during
                # the chain)
                lhsT = grpp.tile([H, 128], BF16, tag="lhsT")
                hsrc = harc[g].ap().rearrange("h (i r) -> h i r", i=GSTEP)
                nc.sync.dma_start(lhsT[:], hsrc[0:H, :, DynSlice(pid * B, B)])
                partials = grpp.tile([128, nmacro], F32, tag="part")
                slabs = [slabp.tile([128, w], BF16, tag="slab",
                                    name=f"slab_{g}_{k}")
                         for k, w in enumerate(pieces)]
                return lhsT, partials, slabs

            def emit_macro(g, m, lhsT, partials, slabs):
                piece, pcol, w = grid[m]
                ps = pmmp.tile([128, MACRO], F32, space="PSUM", tag="mm")
                for c0 in range(0, w, CHUNK):
                    cw = min(CHUNK, w - c0)
                    gcol = piece * PIECE + pcol + c0
                    nc.tensor.matmul(
                        ps[:, c0:c0 + cw], lhsT[:],
                        i2o[:, gcol:gcol + cw],
                        start=True, stop=True)
                nc.scalar.activation(
                    slabs[piece][:, pcol:pcol + w], ps[:, 0:w], AF.Exp,
                    accum_out=partials[:, m:m + 1])

            def finish_group(g, partials, slabs):
                sums_ps = pmmp.tile([4, nmacro], F32, space="PSUM", tag="mm")
                nc.tensor.matmul(sums_ps[:], mask4[:], partials[:],
                                 start=True, stop=True)
                s4 = grpp.tile([4, 1], F32, tag="s4")
                nc.vector.tensor_reduce(s4[:], sums_ps[:],
                                        axis=mybir.AxisListType.X,
                                        op=mybir.AluOpType.add)
                r4 = grpp.tile([4, 1], F32, tag="r4")
                nc.vector.reciprocal(r4[:], s4[:])
                bc_ps = pmmp.tile([128, 1], F32, space="PSUM", tag="mm")
                nc.tensor.matmul(bc_ps[:], maskT4[:], r4[:], start=True, stop=True)
                scal = grpp.tile([128, 1], F32, tag="scal")
                nc.vector.tensor_copy(scal[:], bc_ps[:])

                for k, w in enumerate(pieces):
                    nc.vector.tensor_scalar_mul(slabs[k][:], slabs[k][:],
                                                scal[:, 0:1])
                    nc.sync.dma_start(out_d.ap()[g, :, PIECE * k:PIECE * k + w],
                                      slabs[k][:])

            chain_steps(0, 32)
            g0 = prep_group(0)
            # chain half 2 BEFORE group-0 macros: ACT alternates tanh/exp.
            # Its archive slices land on the sync queue between g0's lhsT
            # load and the piece DMAs, keeping the queue monotone in time.
            chain_steps(32, 64)
            for m in range(nmacro):
                emit_macro(0, m, *g0)
            g1 = prep_group(1)
            finish_group(0, g0[1], g0[2])
            for m in range(nmacro):
                emit_macro(1, m, *g1)
            finish_group(1, g1[1], g1[2])

    nc.compile()
    return nc


_NC_CACHE = None


def _get_nc():
    global _NC_CACHE
    if _NC_CACHE is None:
        _NC_CACHE = build()
    return _NC_CACHE


def _prep_inputs(input_tokens, h0, we, i2h, i2o):
    flat = np.ascontiguousarray(input_tokens, dtype=np.int32).reshape(-1)  # (t,b)
    tokT = np.ascontiguousarray(flat.reshape(16, 128).T)                   # [128,16]
    h0T = np.ascontiguousarray(np.asarray(h0, np.float32).T)               # [16,32]
    we = np.ascontiguousarray(np.asarray(we, np.float32))
    i2h = np.ascontiguousarray(np.asarray(i2h, np.float32))
    i2o = np.asarray(i2o, np.float32)
    i2oB = np.ascontiguousarray(i2o.astype(ml_dtypes.bfloat16))
    mask4 = np.zeros((128, 4), np.float32)
    mask4[np.arange(128), np.arange(128) // 32] = 1.0
    maskT4 = np.ascontiguousarray(mask4.T)
    shared = dict(tokT=tokT, h0T=h0T, we=we, i2h=i2h, i2oB=i2oB,
                  mask4=mask4, maskT4=maskT4)
    return [dict(shared) for _ in range(NCORES)]


def _assemble(results):
    full = np.empty((SEQ, B, V), np.float32)
    for c in range(NCORES):
        o = np.asarray(results[c]["out"]).astype(np.float32)
        o = o.reshape(NGROUP, GSTEP, B, V)
        for g in range(NGROUP):
            for i in range(GSTEP):
                full[c + 32 * g + 8 * i] = o[g, i]
    return full


def run(inputs, trace=False, **kw):
    nc = _get_nc()
    in_maps = _prep_inputs(**inputs)
    res = run_bass_kernel_spmd(nc, in_maps, list(range(NCORES)), trace=trace, **kw)
    return _assemble(res.results), res


def kernel(**inputs):
    out, _ = run(inputs, trace=False)
    return out


# revision 54
# speedup vs baseline: 1.0477x; 1.0477x over previous
"""Trainium2 Bass kernel for the BiRNN LM problem.

Computation (per step t over SEQ=64):
    emb    = we[tok_t]                       [B=32, E=32]
    hidden = tanh([emb, hidden] @ i2h)       [B=32, H=16]
    out_t  = exp(hidden @ i2o)               [B=32, V=32000]
    out_t /= sum(out_t)                      (global sum over the whole slab)

Sharding: sequence dim across 8 cores, interleaved: core c produces output
steps {c + 8k, k=0..7}.  Each step's normalization sum is fully local to one
core => no collectives.  Every core replicates the (tiny) 64-step hidden
recurrence; per-core step selection uses partition_id()-based dynamic DRAM
slices, so all cores run one identical SPMD program.

Layout:
  * EMBH[k] [48, 33*32] f32: rows 0:32 hold transposed embeddings for steps
    32k..32k+31 (col block t-32k), rows 32:48 hold transposed hiddens, with
    h_t landed at col block t+1-32k by the tanh itself.  The recurrence is
    ONE matmul (lhsT=i2h [48,16]) + one tanh per step.
  * Output pass packs 4 steps x 32 batch onto the 128 partitions; the big
    matmul + exp + normalize + store all run in bf16 (except f32 psum/sums),
    which keeps PE at 1 cyc/col, DVE in 2x mode and halves the HBM writes.
    Final upconvert to f32 happens on the host (pure dtype cast).
  * Chain half 2 is emitted before group 0's exp macros: the Tile scheduler
    then naturally alternates tanh (serial chain) with exp macros on ACT,
    hiding half of the chain latency behind group-0 output compute.
"""

import sys
import numpy as np
import ml_dtypes

sys.path.insert(0, "/opt/trn_rl_repo")

import concourse.bass as bass
import concourse.bacc as bacc
import concourse.mybir as mybir
import concourse.tile as tile
from concourse.bass_types import DynSlice
from concourse.bass_utils import run_bass_kernel_spmd
from concourse.tile import add_dep_helper

F32 = mybir.dt.float32
F32R = mybir.dt.float32r
BF16 = mybir.dt.bfloat16
I32 = mybir.dt.int32
AF = mybir.ActivationFunctionType

SEQ, B, E, H, V = 64, 32, 32, 16, 32000
NCORES = 8
NGROUP = 2                   # groups of 4 steps (4*32 = 128 partitions)
GSTEP = 4
CHUNK = 512                  # matmul free dim (one PSUM bank of f32)
MACRO = 1536                 # ACT exp granularity (3 banks)
PIECE = 3072                 # mul + DMA granularity (2 macros)
ILOAD = 4000                 # i2o load slice (cols per DMA)


def build():
    nc = bacc.Bacc("TRN2", target_bir_lowering=False, debug=False,
                   num_devices=NCORES)

    tok_d = nc.dram_tensor("tokT", [128, 16], I32, kind="ExternalInput")
    h0_d = nc.dram_tensor("h0T", [H, B], F32, kind="ExternalInput")
    we_d = nc.dram_tensor("we", [V, E], F32, kind="ExternalInput")
    i2h_d = nc.dram_tensor("i2h", [E + H, H], F32, kind="ExternalInput")
    i2o_d = nc.dram_tensor("i2oB", [H, V], BF16, kind="ExternalInput")
    mask_d = nc.dram_tensor("mask4", [128, 4], F32, kind="ExternalInput")
    maskT_d = nc.dram_tensor("maskT4", [4, 128], F32, kind="ExternalInput")

    out_d = nc.dram_tensor("out", [NGROUP, 128, V], BF16, kind="ExternalOutput")
    # per-half hidden archives (bf16): h of output step t at col 32*(t-32k)
    harc = [nc.dram_tensor(f"harc{k}", [H, 32 * B], BF16, kind="Internal")
            for k in range(2)]

    pieces = [min(PIECE, V - k * PIECE) for k in range((V + PIECE - 1) // PIECE)]
    # macros: (piece, col within piece, width); every macro inside one piece
    grid = []
    col = 0
    while col < V:
        piece = col // PIECE
        w = min(MACRO, V - col, (piece + 1) * PIECE - col)
        grid.append((piece, col - piece * PIECE, w))
        col += w
    nmacro = len(grid)

    with tile.TileContext(nc) as tc:
        with (
            tc.tile_pool(name="const", bufs=1) as constp,
            tc.tile_pool(name="embg", bufs=5) as embgp,
            tc.tile_pool(name="grp", bufs=2) as grpp,
            tc.tile_pool(name="slab", bufs=len(pieces) + 3) as slabp,
            tc.tile_pool(name="pmm", bufs=2, space="PSUM") as pmmp,
            tc.tile_pool(name="phc", bufs=2, space="PSUM") as phcp,
        ):
            pid = nc.partition_id(
                engines=(mybir.EngineType.SP, mybir.EngineType.Pool))

            # dummy activation first: pulls ACT_TABLE_LOAD (exp/tanh set) to
            # t=0 instead of serializing it in front of the first real tanh.
            warm = constp.tile([1, 1], F32)
            nc.vector.memset(warm[:], 0)
            nc.scalar.activation(warm[:], warm[:], AF.Exp)

            # ---- constants / inputs to SBUF ----
            tok = constp.tile([128, 16], I32)
            nc.sync.dma_start(tok[:], tok_d.ap())
            i2h = constp.tile([E + H, H], F32)
            nc.sync.dma_start(i2h[:], i2h_d.ap())
            mask4 = constp.tile([128, 4], F32)
            nc.sync.dma_start(mask4[:], mask_d.ap())
            maskT4 = constp.tile([4, 128], F32)
            nc.sync.dma_start(maskT4[:], maskT_d.ap())

            # combined [emb; h] per half: blocks 0..32, h_t stored at block t+1
            embh = [constp.tile([E + H, 33 * B], F32, name=f"embh{k}")
                    for k in range(2)]
            nc.sync.dma_start(embh[0][E:E + H, 0:B], h0_d.ap())

            # ---- embedding gather + DVE 32x32 block transposes ----
            for j in range(16):
                eg = embgp.tile([128, E], F32, tag="eg")
                nc.gpsimd.indirect_dma_start(
                    out=eg[:], out_offset=None, in_=we_d.ap(),
                    in_offset=bass.IndirectOffsetOnAxis(ap=tok[:, j:j + 1], axis=0))
                for b in range(4):
                    nc.vector.transpose(
                        embh[j // 8][0:E, 128 * (j % 8) + 32 * b:
                                     128 * (j % 8) + 32 * (b + 1)],
                        eg[32 * b:32 * (b + 1), :])

            # bf16 staging copies of h (DVE, off the serial path): hstage[k]
            # col block s holds h of output step 32k+s.  The group archive
            # DMA reads hstage, so it depends only on the tanhs — not on the
            # embedding transposes that share the embh tiles.
            hstage = [constp.tile([H, 32 * B], BF16, name=f"hstage{k}")
                      for k in range(2)]

            def stage_copy(t):
                # after tanh of step t (t = 4a+3): copy steps 4a..4a+3
                k, a = t // 32, (t % 32) // 4
                if k == 0 and a == 7:
                    # h for steps 28..30 in embh[0] blocks 29..31; step 31's
                    # h in embh[1] block 0
                    nc.vector.tensor_copy(hstage[0][:, 28 * B:31 * B],
                                          embh[0][E:E + H, 29 * B:32 * B])
                    nc.vector.tensor_copy(hstage[0][:, 31 * B:32 * B],
                                          embh[1][E:E + H, 0:B])
                else:
                    nc.vector.tensor_copy(
                        hstage[k][:, 4 * a * B:(4 * a + 4) * B],
                        embh[k][E:E + H, (4 * a + 1) * B:(4 * a + 5) * B])

            # i2o [16, V] on partitions 0:16, sliced loads on the scalar
            # HWDGE queue (keeps the sync queue short for the group preps;
            # slices stay ahead of the exp pass)
            i2o = constp.tile([H, V], BF16)
            for c0 in range(0, V, ILOAD):
                cw = min(ILOAD, V - c0)
                nc.scalar.dma_start(i2o[:, c0:c0 + cw],
                                    i2o_d.ap()[:, c0:c0 + cw])

            # ---- recurrence: one matmul + one tanh per step ----
            def chain_steps(t0, t1):
                for t in range(t0, t1):
                    k, b = t // 32, t % 32
                    hp = phcp.tile([H, B], F32, space="PSUM", tag="hps")
                    nc.tensor.matmul(hp[:], i2h[:],
                                     embh[k][:, B * b:B * (b + 1)],
                                     start=True, stop=True)
                    dstk, dstb = (t + 1) // 32, (t + 1) % 32
                    if t == 63:
                        dstk, dstb = 1, 32   # park h_64 in embh[1] block 32
                    nc.scalar.activation(
                        embh[dstk][E:E + H, B * dstb:B * (dstb + 1)],
                        hp[:], AF.Tanh)
                    if t % 4 == 3:
                        stage_copy(t)

            def prep_group(g, after=None):
                # archive this half's hiddens (sync/HWDGE — the gpsimd queue
                # stays free for the gathers), then fetch this core's 4 steps
                # with a dynamic-offset DMA read: h of output step c+32g+8i
                # sits at archive col 32*(c+8i), c = partition_id
                arc = nc.sync.dma_start(harc[g].ap(), hstage[g][:])
                if after is not None:
                    add_dep_helper(arc.ins, after.ins,
                                   reason="keep sync DMA queue monotone")
                lhsT = grpp.tile([H, 128], BF16, tag="lhsT")
                hsrc = harc[g].ap().rearrange("h (i r) -> h i r", i=GSTEP)
                last_load = nc.sync.dma_start(
                    lhsT[:], hsrc[0:H, :, DynSlice(pid * B, B)])
                partials = grpp.tile([128, nmacro], F32, tag="part")
                slabs = [slabp.tile([128, w], BF16, tag="slab",
                                    name=f"slab_{g}_{k}")
                         for k, w in enumerate(pieces)]
                return (lhsT, partials, slabs), last_load

            def emit_macro(g, m, lhsT, partials, slabs):
                piece, pcol, w = grid[m]
                ps = pmmp.tile([128, MACRO], F32, space="PSUM", tag="mm")
                for c0 in range(0, w, CHUNK):
                    cw = min(CHUNK, w - c0)
                    gcol = piece * PIECE + pcol + c0
                    nc.tensor.matmul(
                        ps[:, c0:c0 + cw], lhsT[:],
                        i2o[:, gcol:gcol + cw],
                        start=True, stop=True)
                nc.scalar.activation(
                    slabs[piece][:, pcol:pcol + w], ps[:, 0:w], AF.Exp,
                    accum_out=partials[:, m:m + 1])

            def finish_group(g, partials, slabs):
                sums_ps = pmmp.tile([4, nmacro], F32, space="PSUM", tag="mm")
                nc.tensor.matmul(sums_ps[:], mask4[:], partials[:],
                                 start=True, stop=True)
                s4 = grpp.tile([4, 1], F32, tag="s4")
                nc.vector.tensor_reduce(s4[:], sums_ps[:],
                                        axis=mybir.AxisListType.X,
                                        op=mybir.AluOpType.add)
                r4 = grpp.tile([4, 1], F32, tag="r4")
                nc.vector.reciprocal(r4[:], s4[:])
                bc_ps = pmmp.tile([128, 1], F32, space="PSUM", tag="mm")
                nc.tensor.matmul(bc_ps[:], maskT4[:], r4[:], start=True, stop=True)
                scal = grpp.tile([128, 1], F32, tag="scal")
                nc.vector.tensor_copy(scal[:], bc_ps[:])

                for k, w in enumerate(pieces):
                    nc.vector.tensor_scalar_mul(slabs[k][:], slabs[k][:],
                                                scal[:, 0:1])
                    nc.sync.dma_start(out_d.ap()[g, :, PIECE * k:PIECE * k + w],
                                      slabs[k][:])

            chain_steps(0, 32)
            g0, g0_load = prep_group(0)
            # chain half 2 BEFORE group-0 macros: ACT alternates tanh/exp
            chain_steps(32, 64)
            for m in range(nmacro):
                emit_macro(0, m, *g0)
            # group-1 prep BEFORE group-0's piece DMAs: keeps the sync DMA
            # queue monotone in execution time (no head-of-line blocking)
            g1, _ = prep_group(1, after=g0_load)
            finish_group(0, g0[1], g0[2])
            for m in range(nmacro):
                emit_macro(1, m, *g1)
            finish_group(1, g1[1], g1[2])

    nc.compile()
    return nc


_NC_CACHE = None


def _get_nc():
    global _NC_CACHE
    if _NC_CACHE is None:
        _NC_CACHE = build()
    return _NC_CACHE


def _prep_inputs(input_tokens, h0, we, i2h, i2o):
    flat = np.ascontiguousarray(input_tokens, dtype=np.int32).reshape(-1)  # (t,b)
    tokT = np.ascontiguousarray(flat.reshape(16, 128).T)                   # [128,16]
    h0T = np.ascontiguousarray(np.asarray(h0, np.float32).T)               # [16,32]
    we = np.ascontiguousarray(np.asarray(we, np.float32))
    i2h = np.ascontiguousarray(np.asarray(i2h, np.float32))
    i2o = np.asarray(i2o, np.float32)
    i2oB = np.ascontiguousarray(i2o.astype(ml_dtypes.bfloat16))
    mask4 = np.zeros((128, 4), np.float32)
    mask4[np.arange(128), np.arange(128) // 32] = 1.0
    maskT4 = np.ascontiguousarray(mask4.T)
    shared = dict(tokT=tokT, h0T=h0T, we=we, i2h=i2h, i2oB=i2oB,
                  mask4=mask4, maskT4=maskT4)
    return [dict(shared) for _ in range(NCORES)]


def _assemble(results):
    full = np.empty((SEQ, B, V), np.float32)
    for c in range(NCORES):
        o = np.asarray(results[c]["out"]).astype(np.float32)
        o = o.reshape(NGROUP, GSTEP, B, V)
        for g in range(NGROUP):
            for i in range(GSTEP):
                full[c + 32 * g + 8 * i] = o[g, i]
    return full


def run(inputs, trace=False, **kw):
    nc = _get_nc()
    in_maps = _prep_inputs(**inputs)
    res = run_bass_kernel_spmd(nc, in_maps, list(range(NCORES)), trace=trace, **kw)
    return _assemble(res.results), res


def kernel(**inputs):
    out, _ = run(inputs, trace=False)
    return out
